# revision 33
# baseline (speedup 1.0000x reference)
"""Distributed Trainium2 (8 NeuronCores) GQA attention kernel.

Problem: B=1, T=2048, D=4096, N=32 q-heads, K=8 kv-heads, H=128 (causal,
RMSNorm on q/k/v with (1+scale) on q/k, RoPE base 10000).

Sharding (tensor parallel over heads, per the hint):
  core c owns q-heads [4c, 4c+4) and kv-head c (GQA group preserved, G=4).
  x is replicated (pre-transposed + fp16 on host). Each core computes its
  heads' projections + norms + RoPE + causal attention; per-head attention
  outputs are AllGathered (fp16) as soon as each head finishes, and each
  core computes the final output projection for its own 512-wide slice of
  D. Host concatenates the 8 [2048, 512] f32 slices -> [1, 2048, 4096].
  No partial sums anywhere.

Pipeline: t is processed in 4 groups of 512. Per group: project q/k/v,
run attention per head (AllGather issued immediately per head), and emit
the PREVIOUS group's output projection after this group's attention so
TensorE work always covers collective latency; only the last group's
o-proj is exposed at the tail.

Precision: fp16 storage for x/weights/q/k/out (8x finer mantissa than bf16
at the same byte width), bf16 for exp(logits) and v (needs exponent range:
softmax is computed WITHOUT max subtraction -- max logit ~68, e^68 fits in
bf16/f32 range but not fp16). All matmul accumulation is f32 in PSUM, norms
and softmax math in f32. Measured rel_l2 vs the f32 reference: ~2.5e-3.

Layout trick: logits are computed TRANSPOSED, lT[s,t] = kT.T @ qT, so that
exp(lT) is directly the AV-matmul rhs (no [t,s]->[s,t] transposes of the
2048x2048 softmax matrix). Softmax sum over s (the partition dim) is a
ones-vector matmul on the TensorEngine; 1/Z is partition-broadcast on
GpSimd and folded into the PSUM->SBUF copy of the AV output.
"""

import numpy as np

# ---------------------------------------------------------------- constants
T = 2048          # sequence length
D = 4096          # model dim
H = 128           # head dim
NH = 4            # q heads per core
NHEADS = 32       # total q heads
DC = 32           # d-chunks of 128 (contraction tiles)
TC = 16           # t-chunks of 128
NG = 4            # t-groups of 512 (pipeline granularity)
DSL = 512         # output D slice per core
N_CORES = 8
EPS = 1e-6
ROPE_BASE = 10000.0

_CACHE = {}


# ---------------------------------------------------------------- builder
def _build():
    import concourse.mybir as mybir
    import concourse.tile as tile
    from concourse import bacc
    from concourse.masks import make_identity

    FP16 = mybir.dt.float16
    BF16 = mybir.dt.bfloat16
    F32 = mybir.dt.float32
    Act = mybir.ActivationFunctionType
    Alu = mybir.AluOpType

    nc = bacc.Bacc("TRN2", target_bir_lowering=False, debug=False,
                   num_devices=N_CORES)

    # -------- kernel I/O (per-core shards, preprocessed on host)
    xt_d = nc.dram_tensor("xt", [TC, 128, DC, 128], FP16, kind="ExternalInput")
    wq_d = nc.dram_tensor("wq", [128, DC, NH * 128], FP16, kind="ExternalInput")
    wkv_d = nc.dram_tensor("wkv", [128, DC, 256], FP16, kind="ExternalInput")
    wo_d = nc.dram_tensor("wo", [128, NHEADS, DSL], FP16, kind="ExternalInput")
    cs_d = nc.dram_tensor("csp", [TC, 128, 2 * NH * 64], F32,
                          kind="ExternalInput")
    qsb_d = nc.dram_tensor("qsb", [128, NH * 128], F32, kind="ExternalInput")
    ksb_d = nc.dram_tensor("ksb", [128, 128], F32, kind="ExternalInput")
    maskT_d = nc.dram_tensor("maskt", [128, 128], F32, kind="ExternalInput")
    out_d = nc.dram_tensor("out", [T, DSL], F32, kind="ExternalOutput")

    rg = [list(range(N_CORES))]

    with tile.TileContext(nc) as tc:
        with (
            tc.tile_pool(name="wp", bufs=1) as wp,
            tc.tile_pool(name="xp", bufs=3) as xp,
            tc.tile_pool(name="np_", bufs=2) as np_,
            tc.tile_pool(name="pp", bufs=1) as pp,
            tc.tile_pool(name="op", bufs=2) as op,
            tc.tile_pool(name="ps", bufs=1, space="PSUM") as ps,
            tc.tile_pool(name="dr", bufs=1, space="DRAM") as dr,
        ):
            # -------- resident weights / constants
            # wq/wkv split into pieces so the first projection matmuls only
            # wait on the first 0.5MB; wo is deferred (not needed until the
            # first o-proj, ~1/3 into the kernel).
            wq_sb = wp.tile([128, DC * NH * 128], FP16, tag="wq")
            wq_flat = wq_d.ap().rearrange("p a b -> p (a b)")
            wkv_sb = wp.tile([128, DC * 256], FP16, tag="wkv")
            wkv_flat = wkv_d.ap().rearrange("p a b -> p (a b)")
            wo_sb = wp.tile([128, NHEADS * DSL], FP16, tag="wo")
            qsb_sb = wp.tile([128, NH * 128], F32, tag="qsb")
            ksb_sb = wp.tile([128, 128], F32, tag="ksb")
            maskT_sb = wp.tile([128, 128], F32, tag="maskt")
            ident = wp.tile([128, 128], FP16, tag="ident")
            make_identity(nc, ident[:])
            ones_bf = wp.tile([128, 1], BF16, tag="ones")
            nc.vector.memset(ones_bf[:], 1.0)
            eps_sb = wp.tile([128, 1], F32, tag="eps")
            nc.vector.memset(eps_sb[:], EPS)

            # resident K^T [h, s], V [s, h] (fp16 / bf16), one kv head
            kT_sb = wp.tile([128, T], FP16, tag="kT")
            vf_sb = wp.tile([128, T], BF16, tag="vf")

            def load_chunk(ti):
                """Issue the input DMAs for t-chunk ti (x slab + rope)."""
                xt = xp.tile([128, DC * 128], FP16, tag="xt")
                xt_src = xt_d.ap()[ti].rearrange("p a b -> p (a b)")
                for i in range(4):
                    nc.sync.dma_start(xt[:, i * 1024:(i + 1) * 1024],
                                      xt_src[:, i * 1024:(i + 1) * 1024])
                cs_t = np_.tile([128, 2 * NH * 64], F32, tag="cs", bufs=4)
                nc.sync.dma_start(cs_t[:], cs_d.ap()[ti])
                return xt, cs_t

            def proj_chunk(j, tl, qT, xt, cs_t):
                """Project q/k/v for t-chunk ti, normalize, rope, store."""
                ti = 4 * j + tl

                q_ps = ps.tile([128, 512], F32, tag="qps")
                for dc in range(DC):
                    nc.tensor.matmul(
                        q_ps[:], lhsT=xt[:, dc * 128:(dc + 1) * 128],
                        rhs=wq_sb[:, dc * 512:(dc + 1) * 512],
                        start=(dc == 0), stop=(dc == DC - 1))
                kv_ps = ps.tile([128, 256], F32, tag="kvps")
                for dc in range(DC):
                    nc.tensor.matmul(
                        kv_ps[:], lhsT=xt[:, dc * 128:(dc + 1) * 128],
                        rhs=wkv_sb[:, dc * 256:(dc + 1) * 256],
                        start=(dc == 0), stop=(dc == DC - 1))

                cos_t = cs_t[:, 0:NH * 64]
                sin_t = cs_t[:, NH * 64:2 * NH * 64]

                # ---- Q: rms stats, (1+qs), rope, *rsqrt, fp16
                sqq = np_.tile([128, NH], F32, tag="sqq")
                scr = np_.tile([128, 128], BF16, tag="scr")
                for n in range(NH):
                    nc.scalar.activation(
                        scr[:], q_ps[:, n * 128:(n + 1) * 128], Act.Square,
                        accum_out=sqq[:, n:n + 1])
                rq = np_.tile([128, NH], F32, tag="rq")
                nc.scalar.activation(rq[:], sqq[:], Act.Sqrt,
                                     scale=1.0 / H, bias=eps_sb[:])
                nc.vector.reciprocal(rq[:], rq[:])

                qa = np_.tile([128, 512], F32, tag="qa")
                nc.vector.tensor_mul(out=qa[:], in0=q_ps[:], in1=qsb_sb[:])
                qf = np_.tile([128, 512], F32, tag="qf")
                t1 = np_.tile([128, 256], F32, tag="t1")
                t2 = np_.tile([128, 256], F32, tag="t2")
                qa3 = qa[:].rearrange("p (n h) -> p n h", n=NH)
                qf3 = qf[:].rearrange("p (n h) -> p n h", n=NH)
                c3 = cos_t.rearrange("p (n h) -> p n h", n=NH)
                s3 = sin_t.rearrange("p (n h) -> p n h", n=NH)
                t13 = t1[:].rearrange("p (n h) -> p n h", n=NH)
                t23 = t2[:].rearrange("p (n h) -> p n h", n=NH)
                x1, x2 = qa3[:, :, 0:64], qa3[:, :, 64:128]
                nc.vector.tensor_mul(out=t13, in0=x1, in1=c3)
                nc.vector.tensor_mul(out=t23, in0=x2, in1=s3)
                nc.vector.tensor_tensor(
                    out=qf3[:, :, 0:64], in0=t13, in1=t23, op=Alu.subtract)
                nc.vector.tensor_mul(out=t13, in0=x2, in1=c3)
                nc.vector.tensor_mul(out=t23, in0=x1, in1=s3)
                nc.vector.tensor_tensor(
                    out=qf3[:, :, 64:128], in0=t13, in1=t23, op=Alu.add)
                qb = np_.tile([128, 512], FP16, tag="qb")
                for n in range(NH):
                    nc.vector.tensor_scalar_mul(
                        out=qb[:, n * 128:(n + 1) * 128],
                        in0=qf[:, n * 128:(n + 1) * 128],
                        scalar1=rq[:, n:n + 1])
                for n in range(NH):
                    tp = ps.tile([128, 128], FP16, tag="tp")
                    nc.tensor.transpose(
                        tp[:], qb[:, n * 128:(n + 1) * 128], ident[:])
                    nc.scalar.copy(
                        qT[:, n * 512 + tl * 128: n * 512 + (tl + 1) * 128],
                        tp[:])

                # ---- K: rms stats, (1+ks), rope, fp16, transpose
                sqk = np_.tile([128, 2], F32, tag="sqk")
                nc.scalar.activation(scr[:], kv_ps[:, 0:128], Act.Square,
                                     accum_out=sqk[:, 0:1])
                nc.scalar.activation(scr[:], kv_ps[:, 128:256], Act.Square,
                                     accum_out=sqk[:, 1:2])
                rk = np_.tile([128, 2], F32, tag="rk")
                nc.scalar.activation(rk[:], sqk[:], Act.Sqrt,
                                     scale=1.0 / H, bias=eps_sb[:])
                nc.vector.reciprocal(rk[:], rk[:])

                ka = np_.tile([128, 128], F32, tag="ka")
                nc.vector.tensor_mul(out=ka[:], in0=kv_ps[:, 0:128],
                                     in1=ksb_sb[:])
                kf = np_.tile([128, 128], F32, tag="kf")
                nc.vector.tensor_mul(out=t1[:, 0:64], in0=ka[:, 0:64],
                                     in1=cos_t[:, 0:64])
                nc.vector.tensor_mul(out=t2[:, 0:64], in0=ka[:, 64:128],
                                     in1=sin_t[:, 0:64])
                nc.vector.tensor_tensor(out=kf[:, 0:64], in0=t1[:, 0:64],
                                        in1=t2[:, 0:64], op=Alu.subtract)
                nc.vector.tensor_mul(out=t1[:, 0:64], in0=ka[:, 64:128],
                                     in1=cos_t[:, 0:64])
                nc.vector.tensor_mul(out=t2[:, 0:64], in0=ka[:, 0:64],
                                     in1=sin_t[:, 0:64])
                nc.vector.tensor_tensor(out=kf[:, 64:128], in0=t1[:, 0:64],
                                        in1=t2[:, 0:64], op=Alu.add)
                kb = np_.tile([128, 128], FP16, tag="kb")
                nc.vector.tensor_scalar_mul(out=kb[:], in0=kf[:],
                                            scalar1=rk[:, 0:1])
                tp = ps.tile([128, 128], FP16, tag="tp")
                nc.tensor.transpose(tp[:], kb[:], ident[:])
                nc.scalar.copy(kT_sb[:, ti * 128:(ti + 1) * 128], tp[:])

                # ---- V: rms only, bf16, stays [s, h]
                nc.vector.tensor_scalar_mul(
                    out=vf_sb[:, ti * 128:(ti + 1) * 128],
                    in0=kv_ps[:, 128:256], scalar1=rk[:, 1:2])

            def attn_head(j, n, qT):
                """Causal attention for local head n over t-group j; returns
                the AllGather output buffer [N_CORES, 128, 512]."""
                nk = 4 * (j + 1)  # causal s-chunks for this group
                pT_tiles = []
                for k in range(nk):
                    lt = ps.tile([128, 512], F32, tag="lt", bufs=2)
                    nc.tensor.matmul(
                        lt[:], lhsT=kT_sb[:, k * 128:(k + 1) * 128],
                        rhs=qT[:, n * 512:(n + 1) * 512],
                        start=True, stop=True)
                    pT_k = pp.tile([128, 512], BF16, tag=f"pT{k}")
                    dcol = k - 4 * j
                    if dcol >= 0:
                        # diagonal s-chunk: mask in-block upper triangle;
                        # t-chunks left of it are fully masked
                        if dcol > 0:
                            nc.vector.memset(pT_k[:, 0:dcol * 128], 0.0)
                        nc.vector.tensor_add(
                            out=lt[:, dcol * 128:(dcol + 1) * 128],
                            in0=lt[:, dcol * 128:(dcol + 1) * 128],
                            in1=maskT_sb[:])
                        nc.scalar.activation(
                            pT_k[:, dcol * 128:512],
                            lt[:, dcol * 128:512], Act.Exp)
                    else:
                        nc.scalar.activation(pT_k[:], lt[:], Act.Exp)
                    pT_tiles.append(pT_k)

                z = ps.tile([1, 512], F32, tag="z")
                for k in range(nk):
                    nc.tensor.matmul(z[:], lhsT=ones_bf[:],
                                     rhs=pT_tiles[k][:],
                                     start=(k == 0), stop=(k == nk - 1))
                rz = np_.tile([1, 512], F32, tag="rz")
                nc.vector.reciprocal(rz[:], z[:])
                bz = np_.tile([128, 512], F32, tag="bz")
                nc.gpsimd.partition_broadcast(bz[:], rz[:])

                av = ps.tile([128, 512], F32, tag="av")
                for k in range(nk):
                    nc.tensor.matmul(av[:],
                                     lhsT=vf_sb[:, k * 128:(k + 1) * 128],
                                     rhs=pT_tiles[k][:],
                                     start=(k == 0), stop=(k == nk - 1))
                outT = op.tile([128, 512], FP16, tag="outT")
                nc.vector.tensor_mul(out=outT[:], in0=av[:], in1=bz[:])

                ag_in = dr.tile([128, 512], FP16, tag=f"agin{j}_{n}")
                nc.sync.dma_start(ag_in[:], outT[:])
                ag_out = dr.tile([N_CORES, 128, 512], FP16,
                                 tag=f"agout{j}_{n}", addr_space="Shared")
                nc.gpsimd.collective_compute(
                    "AllGather", Alu.bypass, replica_groups=rg,
                    ins=[ag_in.rearrange("a b -> (a b)")],
                    outs=[ag_out.rearrange("a b c -> (a b c)")])
                return ag_out

            def oproj_slots(ti, ags, slots, o_ps, start, stop):
                for slot in slots:
                    agt = op.tile([128, N_CORES * 128], FP16, tag="agt",
                                  bufs=4, name="agt")
                    nc.sync.dma_start(
                        agt[:].rearrange("p (a b) -> p a b", a=N_CORES),
                        ags[slot].rearrange("c h t -> h c t")
                        [:, :, (ti % 4) * 128:(ti % 4 + 1) * 128])
                    for c8 in range(N_CORES):
                        nhead = 4 * c8 + slot
                        nc.tensor.matmul(
                            o_ps[:],
                            lhsT=agt[:, c8 * 128:(c8 + 1) * 128],
                            rhs=wo_sb[:, nhead * 512:(nhead + 1) * 512],
                            start=(start and slot == slots[0] and c8 == 0),
                            stop=(stop and slot == slots[-1]
                                  and c8 == N_CORES - 1))

            def oproj_chunk(j, tl, ags):
                """Output projection (all 32 global heads -> local D slice)
                for t-chunk tl of group j. ags[n][c] holds core c's
                local head n = global head 4c+n."""
                ti = 4 * j + tl
                o_ps = ps.tile([128, 512], F32, tag="ops")
                oproj_slots(ti, ags, [0, 1, 2, 3], o_ps, True, True)
                o_sb = op.tile([128, 512], F32, tag="osb")
                nc.scalar.copy(o_sb[:], o_ps[:])
                nc.sync.dma_start(
                    out_d.ap()[ti * 128:(ti + 1) * 128, :], o_sb[:])

            def oproj_tail(j, ags):
                """Last group: run slots 0-2 for all 4 chunks first (their
                AllGathers land earlier), then slot 3 as a second PSUM pass
                merged with a DVE add, so the final head's AllGather latency
                is covered by slot 0-2 matmuls."""
                partials = []
                for tl in range(4):
                    o_ps = ps.tile([128, 512], F32, tag="ops")
                    oproj_slots(4 * j + tl, ags, [0, 1, 2], o_ps, True, True)
                    o_sb = op.tile([128, 512], F32, tag="osbp", bufs=4,
                                   name="osbp")
                    nc.scalar.copy(o_sb[:], o_ps[:])
                    partials.append(o_sb)
                for tl in range(4):
                    ti = 4 * j + tl
                    o_ps = ps.tile([128, 512], F32, tag="ops")
                    oproj_slots(ti, ags, [3], o_ps, True, True)
                    o_sb2 = op.tile([128, 512], F32, tag="osb2")
                    nc.vector.tensor_add(out=o_sb2[:], in0=o_ps[:],
                                         in1=partials[tl][:])
                    nc.sync.dma_start(
                        out_d.ap()[ti * 128:(ti + 1) * 128, :], o_sb2[:])

            # -------- software pipeline, interleaved at head granularity:
            # attn(j,n) ; proj(j+1,n) ; oproj(j-1,n) round-robin so no
            # engine queue gets a monolithic phase block.
            chunks = {}
            prev_ags = None
            qT_cur = np_.tile([128, NH * 512], FP16, tag="qT", name="qT")
            # interleaved preload: first x-chunk and first wq pieces lead
            chunks[0] = load_chunk(0)
            for i in range(4):
                nc.sync.dma_start(wq_sb[:, i * 2048:(i + 1) * 2048],
                                  wq_flat[:, i * 2048:(i + 1) * 2048])
            chunks[1] = load_chunk(1)
            for i in range(4, 8):
                nc.sync.dma_start(wq_sb[:, i * 2048:(i + 1) * 2048],
                                  wq_flat[:, i * 2048:(i + 1) * 2048])
            for i in range(4):
                nc.sync.dma_start(wkv_sb[:, i * 2048:(i + 1) * 2048],
                                  wkv_flat[:, i * 2048:(i + 1) * 2048])
            chunks[2] = load_chunk(2)
            nc.sync.dma_start(qsb_sb[:], qsb_d.ap())
            nc.sync.dma_start(ksb_sb[:], ksb_d.ap())
            nc.sync.dma_start(maskT_sb[:], maskT_d.ap())
            for tl in range(4):
                if tl + 3 < TC:
                    chunks[tl + 3] = load_chunk(tl + 3)
                proj_chunk(0, tl, qT_cur, *chunks.pop(tl))
            wo_flat = wo_d.ap().rearrange("p a b -> p (a b)")
            for i in range(4):
                nc.sync.dma_start(wo_sb[:, i * 4096:(i + 1) * 4096],
                                  wo_flat[:, i * 4096:(i + 1) * 4096])
            for j in range(NG):
                qT_next = (np_.tile([128, NH * 512], FP16, tag="qT", name="qT")
                           if j + 1 < NG else None)
                ags = []
                for n in range(NH):
                    if prev_ags is not None:
                        oproj_chunk(j - 1, n, prev_ags)
                    ags.append(attn_head(j, n, qT_cur))
                    if j + 1 < NG:
                        ti = 4 * (j + 1) + n
                        if ti + 3 < TC:
                            chunks[ti + 3] = load_chunk(ti + 3)
                        proj_chunk(j + 1, n, qT_next, *chunks.pop(ti))
                prev_ags = ags
                qT_cur = qT_next
            oproj_tail(NG - 1, prev_ags)

    nc.compile()
    return nc


def _get_nc():
    if "nc" not in _CACHE:
        _CACHE["nc"] = _build()
    return _CACHE["nc"]


# ---------------------------------------------------------------- host prep
def _make_in_maps(x, segment_pos, attn_mask, q_w, kv_w, o_w, q_scale, k_scale):
    x = np.asarray(x, np.float32)
    q_w = np.asarray(q_w, np.float32)
    kv_w = np.asarray(kv_w, np.float32)
    o_w = np.asarray(o_w, np.float32)
    q_scale = np.asarray(q_scale, np.float32)
    k_scale = np.asarray(k_scale, np.float32)
    pos = np.asarray(segment_pos)[0].astype(np.float32)

    x2 = x[0]  # [T, D]
    # xt[ti, p, dc, tl] = x[ti*128+tl, dc*128+p]
    xt = np.ascontiguousarray(
        x2.reshape(TC, 128, DC, 128).transpose(0, 3, 2, 1)).astype(np.float16)

    frac = 2.0 * np.arange(H // 2, dtype=np.float32) / H
    ts_ = (ROPE_BASE ** frac).astype(np.float32)
    sinu = pos[:, None] / ts_[None, :]          # [T, 64]
    csp = np.concatenate([np.tile(np.cos(sinu), (1, NH)),
                          np.tile(np.sin(sinu), (1, NH))],
                         axis=1).astype(np.float32).reshape(
        TC, 128, 2 * NH * 64)

    maskT = np.ascontiguousarray(
        np.asarray(attn_mask, np.float32)[0, :128, :128].T)

    qs_row = np.tile(1.0 + q_scale, NH)                       # [512]
    qsb = np.ascontiguousarray(
        np.broadcast_to(qs_row[None, :], (128, NH * 128))).astype(np.float32)
    ksb = np.ascontiguousarray(
        np.broadcast_to((1.0 + k_scale)[None, :], (128, 128))).astype(
            np.float32)

    in_maps = []
    for c in range(N_CORES):
        qw_c = q_w[NH * c:NH * (c + 1)]           # [4, D, H]
        # wq[p, dc, n*128+h] = qw_c[n, dc*128+p, h]
        wq = np.ascontiguousarray(
            qw_c.transpose(1, 0, 2).reshape(DC, 128, NH * H).transpose(
                1, 0, 2)).astype(np.float16)
        kv_c = kv_w[:, c]                         # [2, D, H]
        wkv = np.ascontiguousarray(
            kv_c.transpose(1, 0, 2).reshape(DC, 128, 2 * H).transpose(
                1, 0, 2)).astype(np.float16)
        # wo[h, n, dsl] = o_w[n, h, c*512 + dsl]
        wo = np.ascontiguousarray(
            o_w[:, :, DSL * c:DSL * (c + 1)].transpose(1, 0, 2)).astype(
                np.float16)
        in_maps.append({
            "xt": xt, "wq": wq, "wkv": wkv, "wo": wo,
            "csp": csp, "qsb": qsb, "ksb": ksb,
            "maskt": maskT,
        })
    return in_maps


def _execute(in_maps, trace=False):
    from concourse import bass_utils
    nc = _get_nc()
    return bass_utils.run_bass_kernel_spmd(
        nc, in_maps, core_ids=list(range(N_CORES)), trace=trace)


# ---------------------------------------------------------------- entry
def kernel(x, segment_pos, attn_mask, q_w, kv_w, o_w, q_scale, k_scale):
    in_maps = _make_in_maps(x, segment_pos, attn_mask, q_w, kv_w, o_w,
                            q_scale, k_scale)
    res = _execute(in_maps, trace=False)
    outs = [np.asarray(res.results[c]["out"]) for c in range(N_CORES)]
    full = np.concatenate(outs, axis=1).astype(np.float32)
    return full[None]


# revision 34
# speedup vs baseline: 1.0827x; 1.0827x over previous
"""Distributed Trainium2 (8 NeuronCores) GQA attention kernel.

Problem: B=1, T=2048, D=4096, N=32 q-heads, K=8 kv-heads, H=128 (causal,
RMSNorm on q/k/v with (1+scale) on q/k, RoPE base 10000).

Sharding (tensor parallel over heads, per the hint):
  core c owns q-heads [4c, 4c+4) and kv-head c (GQA group preserved, G=4).
  x is replicated (pre-transposed + fp16 on host). Each core computes its
  heads' projections + norms + RoPE + causal attention; per-head attention
  outputs are AllGathered (fp16) as soon as each head finishes, and each
  core computes the final output projection for its own 512-wide slice of
  D. Host concatenates the 8 [2048, 512] f32 slices -> [1, 2048, 4096].
  No partial sums anywhere.

Pipeline: t is processed in 4 groups of 512. Per group: project q/k/v,
run attention per head (AllGather issued immediately per head), and emit
the PREVIOUS group's output projection after this group's attention so
TensorE work always covers collective latency; only the last group's
o-proj is exposed at the tail.

Precision: fp16 storage for x/weights/q/k/out (8x finer mantissa than bf16
at the same byte width), bf16 for exp(logits) and v (needs exponent range:
softmax is computed WITHOUT max subtraction -- max logit ~68, e^68 fits in
bf16/f32 range but not fp16). All matmul accumulation is f32 in PSUM, norms
and softmax math in f32. Measured rel_l2 vs the f32 reference: ~2.5e-3.

Layout trick: logits are computed TRANSPOSED, lT[s,t] = kT.T @ qT, so that
exp(lT) is directly the AV-matmul rhs (no [t,s]->[s,t] transposes of the
2048x2048 softmax matrix). Softmax sum over s (the partition dim) is a
ones-vector matmul on the TensorEngine; 1/Z is partition-broadcast on
GpSimd and folded into the PSUM->SBUF copy of the AV output.
"""

import numpy as np

# ---------------------------------------------------------------- constants
T = 2048          # sequence length
D = 4096          # model dim
H = 128           # head dim
NH = 4            # q heads per core
NHEADS = 32       # total q heads
DC = 32           # d-chunks of 128 (contraction tiles)
TC = 16           # t-chunks of 128
NG = 4            # t-groups of 512 (pipeline granularity)
DSL = 512         # output D slice per core
N_CORES = 8
EPS = 1e-6
ROPE_BASE = 10000.0

_CACHE = {}


# ---------------------------------------------------------------- builder
def _build():
    import concourse.mybir as mybir
    import concourse.tile as tile
    from concourse import bacc
    from concourse.masks import make_identity

    FP16 = mybir.dt.float16
    BF16 = mybir.dt.bfloat16
    F32 = mybir.dt.float32
    Act = mybir.ActivationFunctionType
    Alu = mybir.AluOpType

    nc = bacc.Bacc("TRN2", target_bir_lowering=False, debug=False,
                   num_devices=N_CORES)

    # -------- kernel I/O (per-core shards, preprocessed on host)
    xt_d = nc.dram_tensor("xt", [TC, 128, DC, 128], FP16, kind="ExternalInput")
    wq_d = nc.dram_tensor("wq", [128, DC, NH * 128], FP16, kind="ExternalInput")
    wkv_d = nc.dram_tensor("wkv", [128, DC, 256], FP16, kind="ExternalInput")
    wo_d = nc.dram_tensor("wo", [128, NHEADS, DSL], FP16, kind="ExternalInput")
    cs_d = nc.dram_tensor("csp", [TC, 128, 2 * NH * 64], F32,
                          kind="ExternalInput")
    qsb_d = nc.dram_tensor("qsb", [128, NH * 128], F32, kind="ExternalInput")
    ksb_d = nc.dram_tensor("ksb", [128, 128], F32, kind="ExternalInput")
    maskT_d = nc.dram_tensor("maskt", [128, 128], F32, kind="ExternalInput")
    out_d = nc.dram_tensor("out", [T, DSL], F32, kind="ExternalOutput")

    rg = [list(range(N_CORES))]

    with tile.TileContext(nc) as tc:
        with (
            tc.tile_pool(name="wp", bufs=1) as wp,
            tc.tile_pool(name="xp", bufs=3) as xp,
            tc.tile_pool(name="np_", bufs=2) as np_,
            tc.tile_pool(name="pp", bufs=1) as pp,
            tc.tile_pool(name="op", bufs=2) as op,
            tc.tile_pool(name="ps", bufs=1, space="PSUM") as ps,
            tc.tile_pool(name="dr", bufs=1, space="DRAM") as dr,
        ):
            # -------- resident weights / constants
            # wq/wkv split into pieces so the first projection matmuls only
            # wait on the first 0.5MB; wo is deferred (not needed until the
            # first o-proj, ~1/3 into the kernel).
            wq_sb = wp.tile([128, DC * NH * 128], FP16, tag="wq")
            wq_flat = wq_d.ap().rearrange("p a b -> p (a b)")
            wkv_sb = wp.tile([128, DC * 256], FP16, tag="wkv")
            wkv_flat = wkv_d.ap().rearrange("p a b -> p (a b)")
            wo_sb = wp.tile([128, NHEADS * DSL], FP16, tag="wo")
            qsb_sb = wp.tile([128, NH * 128], F32, tag="qsb")
            ksb_sb = wp.tile([128, 128], F32, tag="ksb")
            maskT_sb = wp.tile([128, 128], F32, tag="maskt")
            ident = wp.tile([128, 128], FP16, tag="ident")
            make_identity(nc, ident[:])
            ones_bf = wp.tile([128, 1], BF16, tag="ones")
            nc.vector.memset(ones_bf[:], 1.0)
            eps_sb = wp.tile([128, 1], F32, tag="eps")
            nc.vector.memset(eps_sb[:], EPS)

            # resident K^T [h, s], V [s, h] (fp16 / bf16), one kv head
            kT_sb = wp.tile([128, T], FP16, tag="kT")
            vf_sb = wp.tile([128, T], BF16, tag="vf")

            def load_chunk(ti):
                """Issue the input DMAs for t-chunk ti (x slab + rope)."""
                xt = xp.tile([128, DC * 128], FP16, tag="xt")
                xt_src = xt_d.ap()[ti].rearrange("p a b -> p (a b)")
                for i in range(4):
                    nc.sync.dma_start(xt[:, i * 1024:(i + 1) * 1024],
                                      xt_src[:, i * 1024:(i + 1) * 1024])
                cs_t = np_.tile([128, 2 * NH * 64], F32, tag="cs", bufs=4)
                nc.sync.dma_start(cs_t[:], cs_d.ap()[ti])
                return xt, cs_t

            def proj_chunk(j, tl, qT, xt, cs_t):
                """Project q/k/v for t-chunk ti, normalize, rope, store."""
                ti = 4 * j + tl

                q_ps = ps.tile([128, 512], F32, tag="qps")
                kv_ps = ps.tile([128, 256], F32, tag="kvps")
                for dc in range(DC):
                    nc.tensor.matmul(
                        q_ps[:], lhsT=xt[:, dc * 128:(dc + 1) * 128],
                        rhs=wq_sb[:, dc * 512:(dc + 1) * 512],
                        start=(dc == 0), stop=(dc == DC - 1))
                    nc.tensor.matmul(
                        kv_ps[:], lhsT=xt[:, dc * 128:(dc + 1) * 128],
                        rhs=wkv_sb[:, dc * 256:(dc + 1) * 256],
                        start=(dc == 0), stop=(dc == DC - 1))

                cos_t = cs_t[:, 0:NH * 64]
                sin_t = cs_t[:, NH * 64:2 * NH * 64]

                # ---- Q: rms stats, (1+qs), rope, *rsqrt, fp16
                sqq = np_.tile([128, NH], F32, tag="sqq")
                scr = np_.tile([128, 128], BF16, tag="scr")
                for n in range(NH):
                    nc.scalar.activation(
                        scr[:], q_ps[:, n * 128:(n + 1) * 128], Act.Square,
                        accum_out=sqq[:, n:n + 1])
                rq = np_.tile([128, NH], F32, tag="rq")
                nc.scalar.activation(rq[:], sqq[:], Act.Sqrt,
                                     scale=1.0 / H, bias=eps_sb[:])
                nc.vector.reciprocal(rq[:], rq[:])

                qa = np_.tile([128, 512], F32, tag="qa")
                nc.vector.tensor_mul(out=qa[:], in0=q_ps[:], in1=qsb_sb[:])
                qf = np_.tile([128, 512], F32, tag="qf")
                t1 = np_.tile([128, 256], F32, tag="t1")
                t2 = np_.tile([128, 256], F32, tag="t2")
                qa3 = qa[:].rearrange("p (n h) -> p n h", n=NH)
                qf3 = qf[:].rearrange("p (n h) -> p n h", n=NH)
                c3 = cos_t.rearrange("p (n h) -> p n h", n=NH)
                s3 = sin_t.rearrange("p (n h) -> p n h", n=NH)
                t13 = t1[:].rearrange("p (n h) -> p n h", n=NH)
                t23 = t2[:].rearrange("p (n h) -> p n h", n=NH)
                x1, x2 = qa3[:, :, 0:64], qa3[:, :, 64:128]
                nc.vector.tensor_mul(out=t13, in0=x1, in1=c3)
                nc.vector.tensor_mul(out=t23, in0=x2, in1=s3)
                nc.vector.tensor_tensor(
                    out=qf3[:, :, 0:64], in0=t13, in1=t23, op=Alu.subtract)
                nc.vector.tensor_mul(out=t13, in0=x2, in1=c3)
                nc.vector.tensor_mul(out=t23, in0=x1, in1=s3)
                nc.vector.tensor_tensor(
                    out=qf3[:, :, 64:128], in0=t13, in1=t23, op=Alu.add)
                qb = np_.tile([128, 512], FP16, tag="qb")
                for n in range(NH):
                    nc.vector.tensor_scalar_mul(
                        out=qb[:, n * 128:(n + 1) * 128],
                        in0=qf[:, n * 128:(n + 1) * 128],
                        scalar1=rq[:, n:n + 1])
                for n in range(NH):
                    tp = ps.tile([128, 128], FP16, tag="tp")
                    nc.tensor.transpose(
                        tp[:], qb[:, n * 128:(n + 1) * 128], ident[:])
                    nc.scalar.copy(
                        qT[:, n * 512 + tl * 128: n * 512 + (tl + 1) * 128],
                        tp[:])

                # ---- K: rms stats, (1+ks), rope, fp16, transpose
                sqk = np_.tile([128, 2], F32, tag="sqk")
                nc.scalar.activation(scr[:], kv_ps[:, 0:128], Act.Square,
                                     accum_out=sqk[:, 0:1])
                nc.scalar.activation(scr[:], kv_ps[:, 128:256], Act.Square,
                                     accum_out=sqk[:, 1:2])
                rk = np_.tile([128, 2], F32, tag="rk")
                nc.scalar.activation(rk[:], sqk[:], Act.Sqrt,
                                     scale=1.0 / H, bias=eps_sb[:])
                nc.vector.reciprocal(rk[:], rk[:])

                ka = np_.tile([128, 128], F32, tag="ka")
                nc.vector.tensor_mul(out=ka[:], in0=kv_ps[:, 0:128],
                                     in1=ksb_sb[:])
                kf = np_.tile([128, 128], F32, tag="kf")
                nc.vector.tensor_mul(out=t1[:, 0:64], in0=ka[:, 0:64],
                                     in1=cos_t[:, 0:64])
                nc.vector.tensor_mul(out=t2[:, 0:64], in0=ka[:, 64:128],
                                     in1=sin_t[:, 0:64])
                nc.vector.tensor_tensor(out=kf[:, 0:64], in0=t1[:, 0:64],
                                        in1=t2[:, 0:64], op=Alu.subtract)
                nc.vector.tensor_mul(out=t1[:, 0:64], in0=ka[:, 64:128],
                                     in1=cos_t[:, 0:64])
                nc.vector.tensor_mul(out=t2[:, 0:64], in0=ka[:, 0:64],
                                     in1=sin_t[:, 0:64])
                nc.vector.tensor_tensor(out=kf[:, 64:128], in0=t1[:, 0:64],
                                        in1=t2[:, 0:64], op=Alu.add)
                kb = np_.tile([128, 128], FP16, tag="kb")
                nc.vector.tensor_scalar_mul(out=kb[:], in0=kf[:],
                                            scalar1=rk[:, 0:1])
                tp = ps.tile([128, 128], FP16, tag="tp")
                nc.tensor.transpose(tp[:], kb[:], ident[:])
                nc.scalar.copy(kT_sb[:, ti * 128:(ti + 1) * 128], tp[:])

                # ---- V: rms only, bf16, stays [s, h]
                nc.vector.tensor_scalar_mul(
                    out=vf_sb[:, ti * 128:(ti + 1) * 128],
                    in0=kv_ps[:, 128:256], scalar1=rk[:, 1:2])

            def attn_head(j, n, qT):
                """Causal attention for local head n over t-group j; returns
                the AllGather output buffer [N_CORES, 128, 512]."""
                nk = 4 * (j + 1)  # causal s-chunks for this group
                pT_tiles = []
                for k in range(nk):
                    lt = ps.tile([128, 512], F32, tag="lt", bufs=2)
                    nc.tensor.matmul(
                        lt[:], lhsT=kT_sb[:, k * 128:(k + 1) * 128],
                        rhs=qT[:, n * 512:(n + 1) * 512],
                        start=True, stop=True)
                    pT_k = pp.tile([128, 512], BF16, tag=f"pT{k}")
                    dcol = k - 4 * j
                    if dcol >= 0:
                        # diagonal s-chunk: mask in-block upper triangle;
                        # t-chunks left of it are fully masked
                        if dcol > 0:
                            nc.vector.memset(pT_k[:, 0:dcol * 128], 0.0)
                        nc.vector.tensor_add(
                            out=lt[:, dcol * 128:(dcol + 1) * 128],
                            in0=lt[:, dcol * 128:(dcol + 1) * 128],
                            in1=maskT_sb[:])
                        nc.scalar.activation(
                            pT_k[:, dcol * 128:512],
                            lt[:, dcol * 128:512], Act.Exp)
                    else:
                        nc.scalar.activation(pT_k[:], lt[:], Act.Exp)
                    pT_tiles.append(pT_k)

                z = ps.tile([1, 512], F32, tag="z")
                for k in range(nk):
                    nc.tensor.matmul(z[:], lhsT=ones_bf[:],
                                     rhs=pT_tiles[k][:],
                                     start=(k == 0), stop=(k == nk - 1))
                rz = np_.tile([1, 512], F32, tag="rz")
                nc.vector.reciprocal(rz[:], z[:])
                bz = np_.tile([128, 512], F32, tag="bz")
                nc.gpsimd.partition_broadcast(bz[:], rz[:])

                av = ps.tile([128, 512], F32, tag="av")
                for k in range(nk):
                    nc.tensor.matmul(av[:],
                                     lhsT=vf_sb[:, k * 128:(k + 1) * 128],
                                     rhs=pT_tiles[k][:],
                                     start=(k == 0), stop=(k == nk - 1))
                outT = op.tile([128, 512], FP16, tag="outT")
                nc.vector.tensor_mul(out=outT[:], in0=av[:], in1=bz[:])

                ag_in = dr.tile([128, 512], FP16, tag=f"agin{j}_{n}")
                nc.sync.dma_start(ag_in[:], outT[:])
                ag_out = dr.tile([N_CORES, 128, 512], FP16,
                                 tag=f"agout{j}_{n}", addr_space="Shared")
                nc.gpsimd.collective_compute(
                    "AllGather", Alu.bypass, replica_groups=rg,
                    ins=[ag_in.rearrange("a b -> (a b)")],
                    outs=[ag_out.rearrange("a b c -> (a b c)")])
                return ag_out

            def oproj_slots(ti, ags, slots, o_ps, start, stop):
                for slot in slots:
                    agt = op.tile([128, N_CORES * 128], FP16, tag="agt",
                                  bufs=4, name="agt")
                    nc.sync.dma_start(
                        agt[:].rearrange("p (a b) -> p a b", a=N_CORES),
                        ags[slot].rearrange("c h t -> h c t")
                        [:, :, (ti % 4) * 128:(ti % 4 + 1) * 128])
                    for c8 in range(N_CORES):
                        nhead = 4 * c8 + slot
                        nc.tensor.matmul(
                            o_ps[:],
                            lhsT=agt[:, c8 * 128:(c8 + 1) * 128],
                            rhs=wo_sb[:, nhead * 512:(nhead + 1) * 512],
                            start=(start and slot == slots[0] and c8 == 0),
                            stop=(stop and slot == slots[-1]
                                  and c8 == N_CORES - 1))

            def oproj_chunk(j, tl, ags):
                """Output projection (all 32 global heads -> local D slice)
                for t-chunk tl of group j. ags[n][c] holds core c's
                local head n = global head 4c+n."""
                ti = 4 * j + tl
                o_ps = ps.tile([128, 512], F32, tag="ops")
                oproj_slots(ti, ags, [0, 1, 2, 3], o_ps, True, True)
                o_sb = op.tile([128, 512], F32, tag="osb")
                nc.scalar.copy(o_sb[:], o_ps[:])
                nc.sync.dma_start(
                    out_d.ap()[ti * 128:(ti + 1) * 128, :], o_sb[:])

            def oproj_tail(j, ags):
                """Last group: run slots 0-2 for all 4 chunks first (their
                AllGathers land earlier), then slot 3 as a second PSUM pass
                merged with a DVE add, so the final head's AllGather latency
                is covered by slot 0-2 matmuls."""
                partials = []
                for tl in range(4):
                    o_ps = ps.tile([128, 512], F32, tag="ops")
                    oproj_slots(4 * j + tl, ags, [0, 1, 2], o_ps, True, True)
                    o_sb = op.tile([128, 512], F32, tag="osbp", bufs=4,
                                   name="osbp")
                    nc.scalar.copy(o_sb[:], o_ps[:])
                    partials.append(o_sb)
                for tl in range(4):
                    ti = 4 * j + tl
                    o_ps = ps.tile([128, 512], F32, tag="ops")
                    oproj_slots(ti, ags, [3], o_ps, True, True)
                    o_sb2 = op.tile([128, 512], F32, tag="osb2")
                    nc.vector.tensor_add(out=o_sb2[:], in0=o_ps[:],
                                         in1=partials[tl][:])
                    nc.sync.dma_start(
                        out_d.ap()[ti * 128:(ti + 1) * 128, :], o_sb2[:])

            # -------- software pipeline, interleaved at head granularity:
            # attn(j,n) ; proj(j+1,n) ; oproj(j-1,n) round-robin so no
            # engine queue gets a monolithic phase block.
            chunks = {}
            prev_ags = None
            qT_cur = np_.tile([128, NH * 512], FP16, tag="qT", name="qT")
            # interleaved preload: first x-chunk and first wq pieces lead
            chunks[0] = load_chunk(0)
            for i in range(4):
                nc.sync.dma_start(wq_sb[:, i * 2048:(i + 1) * 2048],
                                  wq_flat[:, i * 2048:(i + 1) * 2048])
            chunks[1] = load_chunk(1)
            for i in range(4, 8):
                nc.sync.dma_start(wq_sb[:, i * 2048:(i + 1) * 2048],
                                  wq_flat[:, i * 2048:(i + 1) * 2048])
            for i in range(4):
                nc.sync.dma_start(wkv_sb[:, i * 2048:(i + 1) * 2048],
                                  wkv_flat[:, i * 2048:(i + 1) * 2048])
            chunks[2] = load_chunk(2)
            nc.sync.dma_start(qsb_sb[:], qsb_d.ap())
            nc.sync.dma_start(ksb_sb[:], ksb_d.ap())
            nc.sync.dma_start(maskT_sb[:], maskT_d.ap())
            for tl in range(4):
                if tl + 3 < TC:
                    chunks[tl + 3] = load_chunk(tl + 3)
                proj_chunk(0, tl, qT_cur, *chunks.pop(tl))
            wo_flat = wo_d.ap().rearrange("p a b -> p (a b)")
            for i in range(4):
                nc.sync.dma_start(wo_sb[:, i * 4096:(i + 1) * 4096],
                                  wo_flat[:, i * 4096:(i + 1) * 4096])
            for j in range(NG):
                qT_next = (np_.tile([128, NH * 512], FP16, tag="qT", name="qT")
                           if j + 1 < NG else None)
                ags = []
                for n in range(NH):
                    if prev_ags is not None:
                        oproj_chunk(j - 1, n, prev_ags)
                    ags.append(attn_head(j, n, qT_cur))
                    if j + 1 < NG:
                        ti = 4 * (j + 1) + n
                        if ti + 3 < TC:
                            chunks[ti + 3] = load_chunk(ti + 3)
                        proj_chunk(j + 1, n, qT_next, *chunks.pop(ti))
                prev_ags = ags
                qT_cur = qT_next
            oproj_tail(NG - 1, prev_ags)

    nc.compile()
    return nc


def _get_nc():
    if "nc" not in _CACHE:
        _CACHE["nc"] = _build()
    return _CACHE["nc"]


# ---------------------------------------------------------------- host prep
def _make_in_maps(x, segment_pos, attn_mask, q_w, kv_w, o_w, q_scale, k_scale):
    x = np.asarray(x, np.float32)
    q_w = np.asarray(q_w, np.float32)
    kv_w = np.asarray(kv_w, np.float32)
    o_w = np.asarray(o_w, np.float32)
    q_scale = np.asarray(q_scale, np.float32)
    k_scale = np.asarray(k_scale, np.float32)
    pos = np.asarray(segment_pos)[0].astype(np.float32)

    x2 = x[0]  # [T, D]
    # xt[ti, p, dc, tl] = x[ti*128+tl, dc*128+p]
    xt = np.ascontiguousarray(
        x2.reshape(TC, 128, DC, 128).transpose(0, 3, 2, 1)).astype(np.float16)

    frac = 2.0 * np.arange(H // 2, dtype=np.float32) / H
    ts_ = (ROPE_BASE ** frac).astype(np.float32)
    sinu = pos[:, None] / ts_[None, :]          # [T, 64]
    csp = np.concatenate([np.tile(np.cos(sinu), (1, NH)),
                          np.tile(np.sin(sinu), (1, NH))],
                         axis=1).astype(np.float32).reshape(
        TC, 128, 2 * NH * 64)

    maskT = np.ascontiguousarray(
        np.asarray(attn_mask, np.float32)[0, :128, :128].T)

    qs_row = np.tile(1.0 + q_scale, NH)                       # [512]
    qsb = np.ascontiguousarray(
        np.broadcast_to(qs_row[None, :], (128, NH * 128))).astype(np.float32)
    ksb = np.ascontiguousarray(
        np.broadcast_to((1.0 + k_scale)[None, :], (128, 128))).astype(
            np.float32)

    in_maps = []
    for c in range(N_CORES):
        qw_c = q_w[NH * c:NH * (c + 1)]           # [4, D, H]
        # wq[p, dc, n*128+h] = qw_c[n, dc*128+p, h]
        wq = np.ascontiguousarray(
            qw_c.transpose(1, 0, 2).reshape(DC, 128, NH * H).transpose(
                1, 0, 2)).astype(np.float16)
        kv_c = kv_w[:, c]                         # [2, D, H]
        wkv = np.ascontiguousarray(
            kv_c.transpose(1, 0, 2).reshape(DC, 128, 2 * H).transpose(
                1, 0, 2)).astype(np.float16)
        # wo[h, n, dsl] = o_w[n, h, c*512 + dsl]
        wo = np.ascontiguousarray(
            o_w[:, :, DSL * c:DSL * (c + 1)].transpose(1, 0, 2)).astype(
                np.float16)
        in_maps.append({
            "xt": xt, "wq": wq, "wkv": wkv, "wo": wo,
            "csp": csp, "qsb": qsb, "ksb": ksb,
            "maskt": maskT,
        })
    return in_maps


def _execute(in_maps, trace=False):
    from concourse import bass_utils
    nc = _get_nc()
    return bass_utils.run_bass_kernel_spmd(
        nc, in_maps, core_ids=list(range(N_CORES)), trace=trace)


# ---------------------------------------------------------------- entry
def kernel(x, segment_pos, attn_mask, q_w, kv_w, o_w, q_scale, k_scale):
    in_maps = _make_in_maps(x, segment_pos, attn_mask, q_w, kv_w, o_w,
                            q_scale, k_scale)
    res = _execute(in_maps, trace=False)
    outs = [np.asarray(res.results[c]["out"]) for c in range(N_CORES)]
    full = np.concatenate(outs, axis=1).astype(np.float32)
    return full[None]
